# revision 1
# baseline (speedup 1.0000x reference)
"""Trainium2 Bass kernel for nn_ChannelAdaptiveNormalization.

Reference computation (per batch):
    src_n = instnorm(src); q = Wq@src_n; k = Wk@instnorm(trg); v = Wv@trg
    attn = softmax(q^T k / sqrt(C))  over t
    mean = attn @ v ; var = relu(attn @ v^2 - mean^2)
    out = sqrt(mean_s[var]) * src_n + mean_s[mean]      (broadcast over time)

Kernel decomposition (all per-core, data-parallel over batch, 2 batches/core):
  * instance-norm is folded into the CxC projection weights (scale columns by
    1/sd, subtract a rank-1 bias) -- normalized activations never materialize.
  * scores are produced TRANSPOSED ([t, s]) so the attn contraction over t
    needs no transposes; softmax uses exp without max subtraction (scores are
    ~N(0,1)); Z (softmax denominators) via a replicated ones-matmul.
  * only column-reductions of mean/var over s are needed, so the full
    mean matrix is reduced on the fly from PSUM; attn@v^2 collapses to
    a tiny matvec with a_u[t] = sum_s attn[t,s].
  * final output is a per-(b,c) affine of raw src: out = A*src + B.
"""

import os
import sys

import numpy as np

if "/opt/trn_rl_repo" not in sys.path:
    sys.path.insert(0, "/opt/trn_rl_repo")

from contextlib import ExitStack

import concourse.bass as bass
import concourse.tile as tile
from concourse import mybir
from concourse.bass_utils import run_bass_kernel_spmd

DT = mybir.dt
ALU = mybir.AluOpType
ACTF = mybir.ActivationFunctionType

N_CORES = 8
B_FULL = 16
B_SH = B_FULL // N_CORES  # 2 batches per core
C = 256
T = 2048
P = 128
NCH = C // P  # 2 channel chunks
NTCH = T // P  # 16 time chunks
EPS = 1e-5


def _build_nc() -> bass.Bass:
    nc = bass.Bass()
    src = nc.declare_dram_parameter("src", [B_SH, C, T], DT.float32, isOutput=False)
    trg = nc.declare_dram_parameter("trg", [B_SH, C, T], DT.float32, isOutput=False)
    wqt = nc.declare_dram_parameter("wqt", [C, C], DT.float32, isOutput=False)
    wkt = nc.declare_dram_parameter("wkt", [C, C], DT.float32, isOutput=False)
    wvt = nc.declare_dram_parameter("wvt", [C, C], DT.float32, isOutput=False)
    out = nc.declare_dram_parameter("out", [B_SH, C, T], DT.float32, isOutput=True)

    with tile.TileContext(nc) as tc:
        with ExitStack() as ctx:
            _build_kernel(ctx, tc, src, trg, wqt, wkt, wvt, out)
    _legalize_waits(nc)
    return nc


def _legalize_waits(nc: bass.Bass):
    """walrus on this toolchain encodes at most ONE sync wait per
    instruction (NEURON_ISA_TPB_EVENTS has a single wait slot and no
    splitting pass runs).  Hoist all but the last wait of every
    instruction into standalone single-wait EventSemaphore instructions
    on the same engine queue, which preserves ordering semantics."""
    # collect all tile-context data semaphores (skip barrier sems)
    all_sems = {}
    for fn in nc.m.functions:
        for blk in fn.blocks:
            for inst in blk.instructions:
                si = getattr(inst, "sync_info", None)
                if si is None:
                    continue
                for w in list(si.on_wait) + list(si.on_update):
                    if not w.ant_name.startswith("barrier"):
                        all_sems[w.id] = w.ant_name

    for fn in nc.m.functions:
        for blk in fn.blocks:
            snapshot = list(blk.instructions)
            for idx in range(len(snapshot) - 1, -1, -1):
                inst = snapshot[idx]
                if type(inst).__name__ == "InstISA" and getattr(inst, "isa_opcode", None) == 176:
                    # EVENT_SEMAPHORE_RANGE_CLEAR: encoding mismatches this
                    # walrus build; replace with per-sem zero-writes.
                    pos = list(blk.instructions).index(inst)
                    blk.instructions.pop(pos)
                    for sid, sname in sorted(all_sems.items()):
                        ev = mybir.InstEventSemaphore(
                            name=nc.get_next_instruction_name(), ins=[], outs=[]
                        )
                        ev.engine = inst.engine
                        ev.sync_info = mybir.SyncInfo(
                            on_wait=[],
                            on_update=[
                                mybir.SyncUpdate(
                                    sync_type="semaphore",
                                    id=sid,
                                    ant_name=sname,
                                    update_mode="sem-wr-imm",
                                    update_value=0,
                                )
                            ],
                        )
                        nc.register_instruction(ev)
                        blk.instructions.insert(pos, ev)
                        pos += 1

    for fn in nc.m.functions:
        for blk in fn.blocks:
            snapshot = list(blk.instructions)
            for idx in range(len(snapshot) - 1, -1, -1):
                inst = snapshot[idx]
                si = getattr(inst, "sync_info", None)
                if si is None or len(si.on_wait) <= 1:
                    continue
                waits = list(si.on_wait)
                evs = []
                for w in waits[:-1]:
                    ev = mybir.InstEventSemaphore(
                        name=nc.get_next_instruction_name(), ins=[], outs=[]
                    )
                    ev.engine = inst.engine
                    ev.sync_info = mybir.SyncInfo(on_wait=[w], on_update=[])
                    nc.register_instruction(ev)
                    evs.append(ev)
                si.on_wait = waits[-1:]
                inst.sync_info = si
                for ev in reversed(evs):
                    blk.instructions.insert(idx, ev)


def _build_kernel(ctx, tc, src, trg, wqt, wkt, wvt, out):
    nc = tc.nc
    ep = ctx.enter_context

    pool_const = ep(tc.tile_pool(name="const", bufs=1))
    pool_src = ep(tc.tile_pool(name="srcio", bufs=2))
    pool_trg = ep(tc.tile_pool(name="trgio", bufs=2))
    pool_bf = ep(tc.tile_pool(name="bfconv", bufs=1))
    pool_qk = ep(tc.tile_pool(name="qk", bufs=2))
    pool_v = ep(tc.tile_pool(name="vpool", bufs=1))
    pool_p = ep(tc.tile_pool(name="ppool", bufs=16))
    pool_zi = ep(tc.tile_pool(name="zipool", bufs=1))
    pool_stat = ep(tc.tile_pool(name="stat", bufs=2))
    pool_wtmp = ep(tc.tile_pool(name="wtmp", bufs=3))
    pool_junk = ep(tc.tile_pool(name="junk", bufs=1))
    pool_stat2 = ep(tc.tile_pool(name="stat2", bufs=2))
    ps_s = ep(tc.tile_pool(name="ps_s", bufs=2, space="PSUM"))
    ps_z = ep(tc.tile_pool(name="ps_z", bufs=2, space="PSUM"))
    pool_out = ep(tc.tile_pool(name="outio", bufs=2))

    # ---- constants / weights (once) ----
    ones_bf = pool_const.tile([P, P], DT.bfloat16, name="ones_bf")
    nc.vector.memset(ones_bf[:], 1.0)

    # weight layout in SBUF: [128 (c within chunk), NCH*C (cchunk-major, d)]
    wq_bf = pool_const.tile([P, NCH * C], DT.bfloat16, name="wq_bf")
    wk_bf = pool_const.tile([P, NCH * C], DT.bfloat16, name="wk_bf")
    wv_bf = pool_const.tile([P, NCH * C], DT.bfloat16, name="wv_bf")
    for w_bf, w_d in ((wq_bf, wqt), (wk_bf, wkt), (wv_bf, wvt)):
        wtmp = pool_wtmp.tile([P, NCH * C], DT.float32, name="wtmp")
        nc.gpsimd.dma_start(
            wtmp[:].rearrange("p (a d) -> p a d", a=NCH),
            w_d[:].rearrange("(a p) d -> p a d", p=P),
        )
        nc.vector.tensor_copy(w_bf[:], wtmp[:])

    for b in range(B_SH):
        # ================= phase 0: load, stats, conversions ==============
        s_f = []
        t_f = []
        for cc in range(NCH):
            sf = pool_src.tile([P, T], DT.float32, name=f"s_f{cc}")
            nc.gpsimd.dma_start(sf[:], src[b, cc * P : (cc + 1) * P, :])
            s_f.append(sf)
            tf = pool_trg.tile([P, T], DT.float32, name=f"t_f{cc}")
            nc.gpsimd.dma_start(tf[:], trg[b, cc * P : (cc + 1) * P, :])
            t_f.append(tf)

        def rowstats(x_f, nm):
            """-> (mean [P,1] f32 AP, inv_sd [P,1] f32 tile) per row over T."""
            bnst = pool_stat.tile([P, 4 * 6], DT.float32, name=f"bnst_{nm}")
            for j in range(4):
                nc.vector.bn_stats(bnst[:, 6 * j : 6 * (j + 1)], x_f[:, 512 * j : 512 * (j + 1)])
            mv = pool_stat.tile([P, 2], DT.float32, name=f"mv_{nm}")
            nc.vector.bn_aggr(mv[:], bnst[:])
            sd = pool_stat.tile([P, 1], DT.float32, name=f"sd_{nm}")
            # sd = sqrt(var_pop * T/(T-1)) + EPS
            nc.scalar.activation(sd[:], mv[:, 1:2], ACTF.Sqrt, scale=float(T) / (T - 1))
            sde = pool_stat.tile([P, 1], DT.float32, name=f"sde_{nm}")
            nc.vector.tensor_scalar_add(sde[:], sd[:], EPS)
            inv = pool_stat.tile([P, 1], DT.float32, name=f"inv_{nm}")
            nc.vector.reciprocal(inv[:], sde[:])
            return mv[:, 0:1], inv

        s_bf, t_bf = [], []
        for cc in range(NCH):
            sb = pool_bf.tile([P, T], DT.bfloat16, name=f"s_bf{cc}")
            nc.gpsimd.tensor_copy(sb[:], s_f[cc][:])
            s_bf.append(sb)
            tb = pool_bf.tile([P, T], DT.bfloat16, name=f"t_bf{cc}")
            nc.gpsimd.tensor_copy(tb[:], t_f[cc][:])
            t_bf.append(tb)

        mean_s, inv_s, mean_t, inv_t = [], [], [], []
        for cc in range(NCH):
            m, i = rowstats(s_bf[cc], f"s{cc}")
            mean_s.append(m)
            inv_s.append(i)
            m, i = rowstats(t_bf[cc], f"t{cc}")
            mean_t.append(m)
            inv_t.append(i)

        # ============ phase 0w: scaled weights + rank-1 biases ============
        # wq_s[c, d] = wqt[c, d] * inv_s[c]  (bf16), same for wk_s with inv_t
        wq_s = pool_stat.tile([P, NCH * C], DT.bfloat16, name="wq_s")
        wk_s = pool_stat.tile([P, NCH * C], DT.bfloat16, name="wk_s")
        mi_s, mi_t = [], []
        for cc in range(NCH):
            nc.vector.tensor_scalar_mul(
                wq_s[:, cc * C : (cc + 1) * C], wq_bf[:, cc * C : (cc + 1) * C], inv_s[cc][:]
            )
            nc.vector.tensor_scalar_mul(
                wk_s[:, cc * C : (cc + 1) * C], wk_bf[:, cc * C : (cc + 1) * C], inv_t[cc][:]
            )
            mis = pool_stat2.tile([P, 1], DT.bfloat16, name=f"mi_s{cc}")
            nc.vector.tensor_tensor(mis[:], mean_s[cc], inv_s[cc][:], ALU.mult)
            mi_s.append(mis)
            mit = pool_stat2.tile([P, 1], DT.bfloat16, name=f"mi_t{cc}")
            nc.vector.tensor_tensor(mit[:], mean_t[cc], inv_t[cc][:], ALU.mult)
            mi_t.append(mit)

        # PE pre-touches: pull cross-engine operand-ready waits off the first
        # real matmuls (MM encoding allows at most 2 sync waits).
        for ap in (s_bf[0], s_bf[1], t_bf[0], t_bf[1]):
            nc.tensor.ldweights(weights=ap[:, 0:P])
        for ap in (wq_s, wk_s):
            nc.tensor.ldweights(weights=ap[:, 0:P])
        for ap in (mi_s[0], mi_s[1], mi_t[0], mi_t[1]):
            nc.tensor.ldweights(weights=ap[:])

        # beta[d] = sum_c w_s[c,d] * (mu[c]*inv[c]); psum [P, NCH] (d-chunk cols)
        negb = []
        for w_s, mi, nm in ((wq_s, mi_s, "q"), (wk_s, mi_t, "k")):
            bps = ps_s.tile([P, NCH], DT.float32, name="sps", tag="sps")
            for dc in range(NCH):
                for cc in range(NCH):
                    nc.tensor.matmul(
                        bps[:, dc : dc + 1],
                        lhsT=w_s[:, cc * C + dc * P : cc * C + (dc + 1) * P],
                        rhs=mi[cc][:],
                        start=(cc == 0),
                        stop=(cc == NCH - 1),
                    )
            nb = pool_stat2.tile([P, NCH], DT.float32, name=f"negb_{nm}")
            nc.vector.tensor_scalar_mul(nb[:], bps[:], -1.0)
            negb.append(nb)
        negbq, negbk = negb

        # ================= phase 1: projections ===========================
        # Qt/Kt: [d, t] bf16 (per d-chunk tiles), bias folded during eviction
        qt_bf, kt_bf = [], []
        for w_s, nb, outl, nm in ((wq_s, negbq, qt_bf, "qt"), (wk_s, negbk, kt_bf, "kt")):
            x_bf = s_bf if nm == "qt" else t_bf
            for dc in range(NCH):
                ot = pool_qk.tile([P, T], DT.bfloat16, name=f"{nm}{dc}")
                for half in range(2):
                    pps = ps_s.tile([P, 1024], DT.float32, name="sps", tag="sps")
                    for cc in range(NCH):
                        for n4 in range(2):
                            nc.tensor.matmul(
                                pps[:, 512 * n4 : 512 * (n4 + 1)],
                                lhsT=w_s[:, cc * C + dc * P : cc * C + (dc + 1) * P],
                                rhs=x_bf[cc][:, 1024 * half + 512 * n4 : 1024 * half + 512 * (n4 + 1)],
                                start=(cc == 0),
                                stop=(cc == NCH - 1),
                            )
                    nc.scalar.activation(
                        ot[:, 1024 * half : 1024 * (half + 1)],
                        pps[:],
                        ACTF.Identity,
                        bias=nb[:, dc : dc + 1],
                        scale=1.0,
                    )
                outl.append(ot)

        # V_T: [t within chunk, tchunk-major d]  (v_bf[p, 256*j + d])
        v_bf = pool_v.tile([P, NTCH * C], DT.bfloat16, name="v_bf")
        v2_bf = pool_v.tile([P, NTCH * C], DT.bfloat16, name="v2_bf")
        for g in range(4):
            vps = ps_s.tile([P, 1024], DT.float32, name="sps", tag="sps")
            for j4 in range(4):
                j = 4 * g + j4
                for cc in range(NCH):
                    nc.tensor.matmul(
                        vps[:, 256 * j4 : 256 * (j4 + 1)],
                        lhsT=t_bf[cc][:, P * j : P * (j + 1)],
                        rhs=wv_bf[:, cc * C : (cc + 1) * C],
                        start=(cc == 0),
                        stop=(cc == NCH - 1),
                    )
            nc.vector.tensor_copy(v_bf[:, 1024 * g : 1024 * (g + 1)], vps[:])
        nc.vector.tensor_mul(v2_bf[:], v_bf[:], v_bf[:])

        # ====== phases 2-4 per s-half: scores^T, exp, Z, normalize, AV ====
        sm_h = pool_stat.tile([P, 2 * NCH], DT.float32, name="sm_h")
        sm2_h = pool_stat.tile([P, 2 * NCH], DT.float32, name="sm2_h")
        a_uh = pool_stat.tile([P, 2 * NTCH], DT.float32, name="a_uh")
        for sh in range(2):
            so = 1024 * sh
            z_ps = ps_z.tile([P, 1024], DT.float32, name="zav", tag="zav")
            p_t = []
            for tch in range(NTCH):
                p = pool_p.tile([P, 1024], DT.bfloat16, name="p")
                sps = ps_s.tile([P, 1024], DT.float32, name="sps", tag="sps")
                for dc in range(NCH):
                    for n2 in range(2):
                        nc.tensor.matmul(
                            sps[:, 512 * n2 : 512 * (n2 + 1)],
                            lhsT=kt_bf[dc][:, P * tch : P * (tch + 1)],
                            rhs=qt_bf[dc][:, so + 512 * n2 : so + 512 * (n2 + 1)],
                            start=(dc == 0),
                            stop=(dc == NCH - 1),
                        )
                nc.scalar.activation(p[:], sps[:], ACTF.Exp, scale=1.0 / 16.0)
                for n2 in range(2):
                    nc.tensor.matmul(
                        z_ps[:, 512 * n2 : 512 * (n2 + 1)],
                        lhsT=ones_bf[:],
                        rhs=p[:, 512 * n2 : 512 * (n2 + 1)],
                        start=(tch == 0),
                        stop=(tch == NTCH - 1),
                        skip_group_check=True,
                    )
                p_t.append(p)

            nc.vector.reciprocal(z_ps[:], z_ps[:])
            zinv_b = pool_zi.tile([P, 1024], DT.bfloat16, name="zinv_b")
            nc.vector.tensor_copy(zinv_b[:], z_ps[:])

            for i, p in enumerate(p_t):
                # out = (P * 1) * zinv (normalize in place); accum -> a_u half
                nc.vector.scalar_tensor_tensor(
                    out=p[:],
                    in0=p[:],
                    scalar=1.0,
                    in1=zinv_b[:],
                    op0=ALU.mult,
                    op1=ALU.mult,
                    accum_out=a_uh[:, NTCH * sh + i : NTCH * sh + i + 1],
                )

            for dc in range(NCH):
                avps = ps_z.tile([P, 1024], DT.float32, name="zav", tag="zav")
                for tch in range(NTCH):
                    for n2 in range(2):
                        nc.tensor.matmul(
                            avps[:, 512 * n2 : 512 * (n2 + 1)],
                            lhsT=v_bf[:, 256 * tch + P * dc : 256 * tch + P * (dc + 1)],
                            rhs=p_t[tch][:, 512 * n2 : 512 * (n2 + 1)],
                            start=(tch == 0),
                            stop=(tch == NTCH - 1),
                        )
                junk = pool_junk.tile([P, 1024], DT.bfloat16, name="junk")
                nc.scalar.activation(
                    junk[:], avps[:], ACTF.Square,
                    accum_out=sm2_h[:, NCH * sh + dc : NCH * sh + dc + 1],
                )
                nc.vector.reduce_sum(
                    sm_h[:, NCH * sh + dc : NCH * sh + dc + 1], avps[:],
                    axis=mybir.AxisListType.X,
                )

        # combine halves
        a_u = pool_stat.tile([P, NTCH], DT.float32, name="a_u")
        nc.vector.tensor_add(a_u[:], a_uh[:, 0:NTCH], a_uh[:, NTCH : 2 * NTCH])
        a_ub = pool_stat2.tile([P, NTCH], DT.bfloat16, name="a_ub")
        nc.vector.tensor_copy(a_ub[:], a_u[:])
        sm = pool_stat.tile([P, NCH], DT.float32, name="sm")
        nc.vector.tensor_add(sm[:], sm_h[:, 0:NCH], sm_h[:, NCH : 2 * NCH])
        sm2 = pool_stat.tile([P, NCH], DT.float32, name="sm2")
        nc.vector.tensor_add(sm2[:], sm2_h[:, 0:NCH], sm2_h[:, NCH : 2 * NCH])

        # attn@v^2 matvec (per-partition result)
        av2_ps = ps_s.tile([P, NCH], DT.float32, name="sps", tag="sps")
        for dc in range(NCH):
            for j in range(NTCH):
                nc.tensor.matmul(
                    av2_ps[:, dc : dc + 1],
                    lhsT=v2_bf[:, 256 * j + P * dc : 256 * j + P * (dc + 1)],
                    rhs=a_ub[:, j : j + 1],
                    start=(j == 0),
                    stop=(j == NTCH - 1),
                )

        # ================= finals + output ================================
        for dc in range(NCH):
            d1 = pool_stat.tile([P, 1], DT.float32, name=f"d1_{dc}")
            nc.vector.tensor_tensor(d1[:], av2_ps[:, dc : dc + 1], sm2[:, dc : dc + 1], ALU.subtract)
            r1 = pool_stat.tile([P, 1], DT.float32, name=f"r1_{dc}")
            nc.vector.tensor_scalar_max(r1[:], d1[:], 0.0)
            stdv = pool_stat.tile([P, 1], DT.float32, name=f"std_{dc}")
            nc.scalar.activation(stdv[:], r1[:], ACTF.Sqrt, scale=1.0 / T)
            av = pool_stat.tile([P, 1], DT.float32, name=f"av_{dc}")
            nc.vector.tensor_tensor(av[:], stdv[:], inv_s[dc][:], ALU.mult)
            musc = pool_stat.tile([P, 1], DT.float32, name=f"musc_{dc}")
            nc.vector.tensor_scalar_mul(musc[:], sm[:, dc : dc + 1], 1.0 / T)
            negms = pool_stat.tile([P, 1], DT.float32, name=f"negms_{dc}")
            nc.vector.tensor_scalar_mul(negms[:], mean_s[dc], -1.0)
            bv = pool_stat.tile([P, 1], DT.float32, name=f"bv_{dc}")
            nc.vector.scalar_tensor_tensor(
                out=bv[:], in0=av[:], scalar=negms[:], in1=musc[:], op0=ALU.mult, op1=ALU.add
            )
            for half in range(2):
                o_sb = pool_out.tile([P, 1024], DT.float32, name="o_sb")
                nc.scalar.activation(
                    o_sb[:],
                    s_f[dc][:, 1024 * half : 1024 * (half + 1)],
                    ACTF.Identity,
                    bias=bv[:],
                    scale=av[:],
                )
                nc.sync.dma_start(
                    out[b, dc * P : (dc + 1) * P, 1024 * half : 1024 * (half + 1)], o_sb[:]
                )


_NC_CACHE = None


def _get_nc():
    global _NC_CACHE
    if _NC_CACHE is None:
        _NC_CACHE = _build_nc()
    return _NC_CACHE


def _run(src, trg, Wq, Wk, Wv, **kwargs):
    src = np.ascontiguousarray(np.asarray(src, dtype=np.float32))
    trg = np.ascontiguousarray(np.asarray(trg, dtype=np.float32))
    wqt = np.ascontiguousarray(np.asarray(Wq, dtype=np.float32).T)
    wkt = np.ascontiguousarray(np.asarray(Wk, dtype=np.float32).T)
    wvt = np.ascontiguousarray(np.asarray(Wv, dtype=np.float32).T)
    nc = _get_nc()
    in_maps = [
        {
            "src": src[i * B_SH : (i + 1) * B_SH],
            "trg": trg[i * B_SH : (i + 1) * B_SH],
            "wqt": wqt,
            "wkt": wkt,
            "wvt": wvt,
        }
        for i in range(N_CORES)
    ]
    res = run_bass_kernel_spmd(nc, in_maps, list(range(N_CORES)), **kwargs)
    outp = np.concatenate([res.results[i]["out"] for i in range(N_CORES)], axis=0)
    return outp.astype(np.float32), res


def kernel(src, trg, Wq, Wk, Wv):
    outp, _ = _run(src, trg, Wq, Wk, Wv)
    return outp



# revision 5
# speedup vs baseline: 1.1775x; 1.1775x over previous
"""Trainium2 Bass kernel for nn_ChannelAdaptiveNormalization.

Reference computation (per batch):
    src_n = instnorm(src); q = Wq@src_n; k = Wk@instnorm(trg); v = Wv@trg
    attn = softmax(q^T k / sqrt(C))  over t
    mean = attn @ v ; var = relu(attn @ v^2 - mean^2)
    out = sqrt(mean_s[var]) * src_n + mean_s[mean]      (broadcast over time)

Kernel decomposition (per-core, data-parallel over batch, 2 batches/core):
  * instance-norm folded into the CxC projection weights (scale columns by
    1/sd, subtract a rank-1 bias) -- normalized activations never materialize.
  * scores produced TRANSPOSED ([t, s]); exp without max subtraction
    (scores ~N(0,1)); Z via replicated ones-matmul.
  * softmax normalization DEFERRED: attn@v runs on the raw exp'd scores (PE
    only depends on the scalar engine), mean U[d,s] is scaled by 1/Z on
    eviction; a_u[t] = sum_s p[t,s]/Z[s] is a trailing DVE pass feeding the
    tiny attn@v^2 matvec.
  * phase-split emission: loads + casts + stats + weight-scaling for BOTH
    batches are emitted up front so batch 1's prologue hides under batch 0's
    attention; all tile tags are double-buffered across batches.
  * engine placement keeps the scalar engine on the exp_and_others table
    through the attention phases (sqrt only at stats/finals boundaries).
"""

import os
import sys

import numpy as np

if "/opt/trn_rl_repo" not in sys.path:
    sys.path.insert(0, "/opt/trn_rl_repo")

from contextlib import ExitStack

import concourse.bass as bass
import concourse.tile as tile
from concourse import mybir
from concourse.bass_utils import run_bass_kernel_spmd

DT = mybir.dt
ALU = mybir.AluOpType
ACTF = mybir.ActivationFunctionType

N_CORES = 8
B_FULL = 16
B_SH = B_FULL // N_CORES  # 2 batches per core
C = 256
T = 2048
P = 128
NCH = C // P  # 2 channel chunks
NTCH = T // P  # 16 time chunks
EPS = 1e-5


def _build_nc() -> bass.Bass:
    nc = bass.Bass()
    src = nc.declare_dram_parameter("src", [B_SH, C, T], DT.float32, isOutput=False)
    trg = nc.declare_dram_parameter("trg", [B_SH, C, T], DT.float32, isOutput=False)
    wqt = nc.declare_dram_parameter("wqt", [C, C], DT.float32, isOutput=False)
    wkt = nc.declare_dram_parameter("wkt", [C, C], DT.float32, isOutput=False)
    wvt = nc.declare_dram_parameter("wvt", [C, C], DT.float32, isOutput=False)
    out = nc.declare_dram_parameter("out", [B_SH, C, T], DT.float32, isOutput=True)

    with tile.TileContext(nc) as tc:
        with ExitStack() as ctx:
            _build_kernel(ctx, tc, src, trg, wqt, wkt, wvt, out)
    _legalize_waits(nc)
    return nc


def _legalize_waits(nc: bass.Bass):
    """walrus on this toolchain encodes at most ONE sync wait per
    instruction (NEURON_ISA_TPB_EVENTS has a single wait slot and no
    splitting pass runs).  Hoist all but the last wait of every
    instruction into standalone single-wait EventSemaphore instructions
    on the same engine queue, which preserves ordering semantics."""
    # collect all tile-context data semaphores (skip barrier sems)
    all_sems = {}
    for fn in nc.m.functions:
        for blk in fn.blocks:
            for inst in blk.instructions:
                si = getattr(inst, "sync_info", None)
                if si is None:
                    continue
                for w in list(si.on_wait) + list(si.on_update):
                    if not w.ant_name.startswith("barrier"):
                        all_sems[w.id] = w.ant_name

    for fn in nc.m.functions:
        for blk in fn.blocks:
            snapshot = list(blk.instructions)
            for idx in range(len(snapshot) - 1, -1, -1):
                inst = snapshot[idx]
                if type(inst).__name__ == "InstISA" and getattr(inst, "isa_opcode", None) == 176:
                    # EVENT_SEMAPHORE_RANGE_CLEAR: encoding mismatches this
                    # walrus build; replace with per-sem zero-writes.
                    pos = list(blk.instructions).index(inst)
                    blk.instructions.pop(pos)
                    for sid, sname in sorted(all_sems.items()):
                        ev = mybir.InstEventSemaphore(
                            name=nc.get_next_instruction_name(), ins=[], outs=[]
                        )
                        ev.engine = inst.engine
                        ev.sync_info = mybir.SyncInfo(
                            on_wait=[],
                            on_update=[
                                mybir.SyncUpdate(
                                    sync_type="semaphore",
                                    id=sid,
                                    ant_name=sname,
                                    update_mode="sem-wr-imm",
                                    update_value=0,
                                )
                            ],
                        )
                        nc.register_instruction(ev)
                        blk.instructions.insert(pos, ev)
                        pos += 1

    for fn in nc.m.functions:
        for blk in fn.blocks:
            snapshot = list(blk.instructions)
            for idx in range(len(snapshot) - 1, -1, -1):
                inst = snapshot[idx]
                si = getattr(inst, "sync_info", None)
                if si is None or len(si.on_wait) <= 1:
                    continue
                waits = list(si.on_wait)
                evs = []
                for w in waits[:-1]:
                    ev = mybir.InstEventSemaphore(
                        name=nc.get_next_instruction_name(), ins=[], outs=[]
                    )
                    ev.engine = inst.engine
                    ev.sync_info = mybir.SyncInfo(on_wait=[w], on_update=[])
                    nc.register_instruction(ev)
                    evs.append(ev)
                si.on_wait = waits[-1:]
                inst.sync_info = si
                for ev in reversed(evs):
                    blk.instructions.insert(idx, ev)


def _build_kernel(ctx, tc, src, trg, wqt, wkt, wvt, out):
    nc = tc.nc
    ep = ctx.enter_context

    sb = ep(tc.tile_pool(name="sb", bufs=1))
    ps = ep(tc.tile_pool(name="ps", bufs=2, space="PSUM"))

    # ---- constants / weights (once) ----
    ones_bf = sb.tile([P, P], DT.bfloat16, name="ones_bf", tag="ones")
    nc.vector.memset(ones_bf[:], 1.0)

    # weight layout in SBUF: [128 (c within chunk), NCH*C (cchunk-major, d)]
    wq_bf = sb.tile([P, NCH * C], DT.bfloat16, name="wq_bf", tag="wq")
    wk_bf = sb.tile([P, NCH * C], DT.bfloat16, name="wk_bf", tag="wk")
    wv_bf = sb.tile([P, NCH * C], DT.bfloat16, name="wv_bf", tag="wv")
    for w_bf, w_d in ((wq_bf, wqt), (wk_bf, wkt), (wv_bf, wvt)):
        wtmp = sb.tile([P, NCH * C], DT.float32, name="wtmp", tag="wtmp", bufs=2)
        nc.gpsimd.dma_start(
            wtmp[:].rearrange("p (a d) -> p a d", a=NCH),
            w_d[:].rearrange("(a p) d -> p a d", p=P),
        )
        nc.vector.tensor_copy(w_bf[:], wtmp[:])

    # =================================================================
    # PHASE A (both batches): load, cast, stats, scaled weights, betas
    # =================================================================
    s_bf = [[None] * NCH for _ in range(B_SH)]
    t_bf = [[None] * NCH for _ in range(B_SH)]
    mean_s = [[None] * NCH for _ in range(B_SH)]
    inv_s = [[None] * NCH for _ in range(B_SH)]
    wq_s, wk_s = [None] * B_SH, [None] * B_SH
    mi_s = [[None] * NCH for _ in range(B_SH)]
    mi_t = [[None] * NCH for _ in range(B_SH)]
    negb = [[None, None] for _ in range(B_SH)]  # [b][0]=q, [b][1]=k

    def rowstats(b, x_bf, nm):
        """-> (mean [P,1] f32 AP, inv_sd [P,1] f32 tile) per row over T."""
        bnst = sb.tile([P, 4 * 6], DT.float32, name=f"bnst_{nm}", tag="bnst", bufs=4)
        for j in range(4):
            nc.vector.bn_stats(bnst[:, 6 * j : 6 * (j + 1)], x_bf[:, 512 * j : 512 * (j + 1)])
        mv = sb.tile([P, 2], DT.float32, name=f"mv_{nm}", tag=f"mv_{nm}", bufs=2)
        nc.vector.bn_aggr(mv[:], bnst[:])
        sd = sb.tile([P, 1], DT.float32, name=f"sd_{nm}", tag=f"sd_{nm}", bufs=2)
        # sd = sqrt(var_pop * T/(T-1)) + EPS   (sqrt on scalar: phase A only,
        # so the sqrt table is live before the first exp)
        nc.scalar.activation(sd[:], mv[:, 1:2], ACTF.Sqrt, scale=float(T) / (T - 1))
        sde = sb.tile([P, 1], DT.float32, name=f"sde_{nm}", tag=f"sde_{nm}", bufs=2)
        nc.vector.tensor_scalar_add(sde[:], sd[:], EPS)
        inv = sb.tile([P, 1], DT.float32, name=f"inv_{nm}", tag=f"inv_{nm}", bufs=2)
        nc.vector.reciprocal(inv[:], sde[:])
        return mv[:, 0:1], inv

    for b in range(B_SH):
        # staged fp32 loads rotate through one shared tag; trg first so the
        # v/k projection path unblocks earliest.
        for tiles, dram, eng in ((t_bf, trg, "t"), (s_bf, src, "s")):
            for cc in range(NCH):
                xb = sb.tile(
                    [P, T], DT.bfloat16, name=f"{eng}_bf{b}_{cc}", tag=f"{eng}bf{cc}", bufs=2
                )
                tiles[b][cc] = xb
                for h in range(2):
                    stg = sb.tile([P, 1024], DT.float32, name="stg", tag="stage", bufs=6)
                    nc.gpsimd.dma_start(
                        stg[:], dram[b, cc * P : (cc + 1) * P, 1024 * h : 1024 * (h + 1)]
                    )
                    if eng == "t":
                        # trg casts on the scalar engine (Identity)
                        nc.scalar.activation(
                            xb[:, 1024 * h : 1024 * (h + 1)], stg[:], ACTF.Identity
                        )
                    else:
                        nc.gpsimd.tensor_copy(xb[:, 1024 * h : 1024 * (h + 1)], stg[:])

        mean_t_b, inv_t_b = [], []
        for cc in range(NCH):
            m, i = rowstats(b, t_bf[b][cc], f"t{cc}")
            mean_t_b.append(m)
            inv_t_b.append(i)
        for cc in range(NCH):
            m, i = rowstats(b, s_bf[b][cc], f"s{cc}")
            mean_s[b][cc] = m
            inv_s[b][cc] = i

        # scaled weights + rank-1 biases
        wqs = sb.tile([P, NCH * C], DT.bfloat16, name="wq_s", tag="wqs", bufs=2)
        wks = sb.tile([P, NCH * C], DT.bfloat16, name="wk_s", tag="wks", bufs=2)
        wq_s[b], wk_s[b] = wqs, wks
        for cc in range(NCH):
            nc.vector.tensor_scalar_mul(
                wqs[:, cc * C : (cc + 1) * C], wq_bf[:, cc * C : (cc + 1) * C], inv_s[b][cc][:]
            )
            nc.vector.tensor_scalar_mul(
                wks[:, cc * C : (cc + 1) * C], wk_bf[:, cc * C : (cc + 1) * C], inv_t_b[cc][:]
            )
            mis = sb.tile([P, 1], DT.bfloat16, name=f"mi_s{cc}", tag=f"mis{cc}", bufs=2)
            nc.vector.tensor_tensor(mis[:], mean_s[b][cc], inv_s[b][cc][:], ALU.mult)
            mi_s[b][cc] = mis
            mit = sb.tile([P, 1], DT.bfloat16, name=f"mi_t{cc}", tag=f"mit{cc}", bufs=2)
            nc.vector.tensor_tensor(mit[:], mean_t_b[cc], inv_t_b[cc][:], ALU.mult)
            mi_t[b][cc] = mit

        # PE pre-touches: pull cross-engine operand-ready waits off the first
        # real matmuls.
        for ap in (wqs, wks):
            nc.tensor.ldweights(weights=ap[:, 0:P])
        for cc in range(NCH):
            nc.tensor.ldweights(weights=mi_s[b][cc][:])
            nc.tensor.ldweights(weights=mi_t[b][cc][:])

        # beta[d] = sum_c w_s[c,d] * (mu[c]*inv[c]); psum [P, NCH]
        for wi, (w_s, mi, nm) in enumerate(
            ((wqs, mi_s[b], "q"), (wks, mi_t[b], "k"))
        ):
            bps = ps.tile([P, NCH], DT.float32, name="sps", tag="sps")
            for dc in range(NCH):
                for cc in range(NCH):
                    nc.tensor.matmul(
                        bps[:, dc : dc + 1],
                        lhsT=w_s[:, cc * C + dc * P : cc * C + (dc + 1) * P],
                        rhs=mi[cc][:],
                        start=(cc == 0),
                        stop=(cc == NCH - 1),
                    )
            nb = sb.tile([P, NCH], DT.float32, name=f"negb_{nm}", tag=f"negb{nm}", bufs=2)
            nc.vector.tensor_scalar_mul(nb[:], bps[:], -1.0)
            negb[b][wi] = nb

    # =================================================================
    # PHASE B (per batch): projections, attention, finals, output
    # =================================================================
    finals_work = []
    for b in range(B_SH):
        negbq, negbk = negb[b]

        for cc in range(NCH):
            nc.tensor.ldweights(weights=t_bf[b][cc][:, 0:P])
            nc.tensor.ldweights(weights=s_bf[b][cc][:, 0:P])

        # ---- V_T: [t within chunk, tchunk-major d]  (v_bf[p, 256*j + d])
        v_bf = sb.tile([P, NTCH * C], DT.bfloat16, name="v_bf", tag="vbf", bufs=2)
        v2_bf = sb.tile([P, NTCH * C], DT.bfloat16, name="v2_bf", tag="v2bf", bufs=2)
        for g in range(4):
            vps = ps.tile([P, 1024], DT.float32, name="sps", tag="sps")
            for j4 in range(4):
                j = 4 * g + j4
                for cc in range(NCH):
                    nc.tensor.matmul(
                        vps[:, 256 * j4 : 256 * (j4 + 1)],
                        lhsT=t_bf[b][cc][:, P * j : P * (j + 1)],
                        rhs=wv_bf[:, cc * C : (cc + 1) * C],
                        start=(cc == 0),
                        stop=(cc == NCH - 1),
                    )
            nc.scalar.activation(v_bf[:, 1024 * g : 1024 * (g + 1)], vps[:], ACTF.Identity)
        for h in range(2):
            nc.vector.tensor_mul(
                v2_bf[:, 2048 * h : 2048 * (h + 1)],
                v_bf[:, 2048 * h : 2048 * (h + 1)],
                v_bf[:, 2048 * h : 2048 * (h + 1)],
            )

        # ---- Qt/Kt: [d, t] bf16 (per d-chunk tiles), bias folded on eviction
        qt_bf, kt_bf = [], []
        for w_s, nb, outl, nm, x_bf in (
            (wk_s[b], negbk, kt_bf, "kt", t_bf[b]),
            (wq_s[b], negbq, qt_bf, "qt", s_bf[b]),
        ):
            for dc in range(NCH):
                ot = sb.tile([P, T], DT.bfloat16, name=f"{nm}{dc}", tag=f"{nm}{dc}", bufs=2)
                for half in range(2):
                    pps = ps.tile([P, 1024], DT.float32, name="sps", tag="sps")
                    for cc in range(NCH):
                        for n4 in range(2):
                            nc.tensor.matmul(
                                pps[:, 512 * n4 : 512 * (n4 + 1)],
                                lhsT=w_s[:, cc * C + dc * P : cc * C + (dc + 1) * P],
                                rhs=x_bf[cc][:, 1024 * half + 512 * n4 : 1024 * half + 512 * (n4 + 1)],
                                start=(cc == 0),
                                stop=(cc == NCH - 1),
                            )
                    nc.scalar.activation(
                        ot[:, 1024 * half : 1024 * (half + 1)],
                        pps[:],
                        ACTF.Identity,
                        bias=nb[:, dc : dc + 1],
                        scale=1.0,
                    )
                outl.append(ot)

        # ---- attention per s-half: scores^T, exp, Z, raw AV, deferred norm
        sm_h = sb.tile([P, 2 * NCH], DT.float32, name="sm_h", tag="smh", bufs=2)
        sm2_h = sb.tile([P, 2 * NCH], DT.float32, name="sm2_h", tag="sm2h", bufs=2)
        a_uh = sb.tile([P, 2 * NTCH], DT.float32, name="a_uh", tag="auh", bufs=2)
        for sh in range(2):
            so = 1024 * sh
            z_ps = ps.tile([P, 1024], DT.float32, name="zav", tag="zav")
            p_t = []
            for tch in range(NTCH):
                p = sb.tile([P, 1024], DT.bfloat16, name="p", tag="p", bufs=16)
                sps = ps.tile([P, 1024], DT.float32, name="sps", tag="sps")
                for dc in range(NCH):
                    for n2 in range(2):
                        nc.tensor.matmul(
                            sps[:, 512 * n2 : 512 * (n2 + 1)],
                            lhsT=kt_bf[dc][:, P * tch : P * (tch + 1)],
                            rhs=qt_bf[dc][:, so + 512 * n2 : so + 512 * (n2 + 1)],
                            start=(dc == 0),
                            stop=(dc == NCH - 1),
                        )
                nc.scalar.activation(p[:], sps[:], ACTF.Exp, scale=1.0 / 16.0)
                for n2 in range(2):
                    nc.tensor.matmul(
                        z_ps[:, 512 * n2 : 512 * (n2 + 1)],
                        lhsT=ones_bf[:],
                        rhs=p[:, 512 * n2 : 512 * (n2 + 1)],
                        start=(tch == 0),
                        stop=(tch == NTCH - 1),
                        skip_group_check=True,
                    )
                p_t.append(p)

            # zinv = exp(-ln(Z)) on the scalar engine: ln and exp share the
            # natural_log_exp_and_others activation table (no swap), and this
            # keeps the slow DVE reciprocal off the [128,1024] tile.
            zln = sb.tile([P, 1024], DT.float32, name="zln", tag="zln", bufs=2)
            nc.scalar.activation(zln[:], z_ps[:], ACTF.Ln)
            zinv_f = sb.tile([P, 1024], DT.float32, name="zinv_f", tag="zinv", bufs=2)
            nc.scalar.activation(zinv_f[:], zln[:], ACTF.Exp, scale=-1.0)

            # trailing a_u pass on DVE (frees p tiles for the next half);
            # the AV matmuls below read the raw p tiles concurrently.
            for i, p in enumerate(p_t):
                junk = sb.tile([P, 1024], DT.bfloat16, name="junk", tag="junk", bufs=2)
                nc.vector.scalar_tensor_tensor(
                    out=junk[:],
                    in0=p[:],
                    scalar=1.0,
                    in1=zinv_f[:],
                    op0=ALU.mult,
                    op1=ALU.mult,
                    accum_out=a_uh[:, NTCH * sh + i : NTCH * sh + i + 1],
                )

            for dc in range(NCH):
                avps = ps.tile([P, 1024], DT.float32, name="zav", tag="zav")
                for tch in range(NTCH):
                    for n2 in range(2):
                        nc.tensor.matmul(
                            avps[:, 512 * n2 : 512 * (n2 + 1)],
                            lhsT=v_bf[:, 256 * tch + P * dc : 256 * tch + P * (dc + 1)],
                            rhs=p_t[tch][:, 512 * n2 : 512 * (n2 + 1)],
                            start=(tch == 0),
                            stop=(tch == NTCH - 1),
                        )
                # mean[s,d] = U[d,s] * zinv[s]; accumulate sum_s on the fly
                un = sb.tile([P, 1024], DT.bfloat16, name="un", tag="un", bufs=2)
                nc.vector.scalar_tensor_tensor(
                    out=un[:],
                    in0=avps[:],
                    scalar=1.0,
                    in1=zinv_f[:],
                    op0=ALU.mult,
                    op1=ALU.mult,
                    accum_out=sm_h[:, NCH * sh + dc : NCH * sh + dc + 1],
                )
                junksq = sb.tile([P, 1024], DT.bfloat16, name="junksq", tag="junksq", bufs=2)
                nc.scalar.activation(
                    junksq[:], un[:], ACTF.Square,
                    accum_out=sm2_h[:, NCH * sh + dc : NCH * sh + dc + 1],
                )

        # ---- combine halves + attn@v^2 matvec
        a_u = sb.tile([P, NTCH], DT.float32, name="a_u", tag="au", bufs=2)
        nc.vector.tensor_add(a_u[:], a_uh[:, 0:NTCH], a_uh[:, NTCH : 2 * NTCH])
        a_ub = sb.tile([P, NTCH], DT.bfloat16, name="a_ub", tag="aub", bufs=2)
        nc.vector.tensor_copy(a_ub[:], a_u[:])
        sm = sb.tile([P, NCH], DT.float32, name="sm", tag="sm", bufs=2)
        nc.vector.tensor_add(sm[:], sm_h[:, 0:NCH], sm_h[:, NCH : 2 * NCH])
        sm2 = sb.tile([P, NCH], DT.float32, name="sm2", tag="sm2", bufs=2)
        nc.vector.tensor_add(sm2[:], sm2_h[:, 0:NCH], sm2_h[:, NCH : 2 * NCH])

        av2_ps = ps.tile([P, NCH], DT.float32, name="sps", tag="sps")
        for dc in range(NCH):
            for j in range(NTCH):
                nc.tensor.matmul(
                    av2_ps[:, dc : dc + 1],
                    lhsT=v2_bf[:, 256 * j + P * dc : 256 * j + P * (dc + 1)],
                    rhs=a_ub[:, j : j + 1],
                    start=(j == 0),
                    stop=(j == NTCH - 1),
                )
        av2 = sb.tile([P, NCH], DT.float32, name="av2", tag="av2", bufs=2)
        nc.vector.tensor_copy(av2[:], av2_ps[:])

        # ---- finals + output
        for dc in range(NCH):
            d1 = sb.tile([P, 1], DT.float32, name=f"d1_{dc}", tag=f"d1{dc}", bufs=2)
            nc.vector.tensor_tensor(d1[:], av2[:, dc : dc + 1], sm2[:, dc : dc + 1], ALU.subtract)
            r1 = sb.tile([P, 1], DT.float32, name=f"r1_{dc}", tag=f"r1{dc}", bufs=2)
            nc.vector.tensor_scalar_max(r1[:], d1[:], 0.0)
            stdv = sb.tile([P, 1], DT.float32, name=f"std_{dc}", tag=f"std{dc}", bufs=2)
            nc.scalar.activation(stdv[:], r1[:], ACTF.Sqrt, scale=1.0 / T)
            av = sb.tile([P, 1], DT.float32, name=f"av_{dc}", tag=f"av{dc}", bufs=2)
            nc.vector.tensor_tensor(av[:], stdv[:], inv_s[b][dc][:], ALU.mult)
            musc = sb.tile([P, 1], DT.float32, name=f"musc_{dc}", tag=f"musc{dc}", bufs=2)
            nc.vector.tensor_scalar_mul(musc[:], sm[:, dc : dc + 1], 1.0 / T)
            negms = sb.tile([P, 1], DT.float32, name=f"negms_{dc}", tag=f"negms{dc}", bufs=2)
            nc.vector.tensor_scalar_mul(negms[:], mean_s[b][dc], -1.0)
            bv = sb.tile([P, 1], DT.float32, name=f"bv_{dc}", tag=f"bv{dc}", bufs=2)
            nc.vector.scalar_tensor_tensor(
                out=bv[:], in0=av[:], scalar=negms[:], in1=musc[:], op0=ALU.mult, op1=ALU.add
            )
            for half in range(2):
                o_sb = sb.tile([P, 1024], DT.float32, name="o_sb", tag="osb", bufs=3)
                nc.scalar.activation(
                    o_sb[:],
                    s_bf[b][dc][:, 1024 * half : 1024 * (half + 1)],
                    ACTF.Identity,
                    bias=bv[:],
                    scale=av[:],
                )
                nc.sync.dma_start(
                    out[b, dc * P : (dc + 1) * P, 1024 * half : 1024 * (half + 1)], o_sb[:]
                )


_NC_CACHE = None


def _get_nc():
    global _NC_CACHE
    if _NC_CACHE is None:
        _NC_CACHE = _build_nc()
    return _NC_CACHE


def _run(src, trg, Wq, Wk, Wv, **kwargs):
    src = np.ascontiguousarray(np.asarray(src, dtype=np.float32))
    trg = np.ascontiguousarray(np.asarray(trg, dtype=np.float32))
    wqt = np.ascontiguousarray(np.asarray(Wq, dtype=np.float32).T)
    wkt = np.ascontiguousarray(np.asarray(Wk, dtype=np.float32).T)
    wvt = np.ascontiguousarray(np.asarray(Wv, dtype=np.float32).T)
    nc = _get_nc()
    in_maps = [
        {
            "src": src[i * B_SH : (i + 1) * B_SH],
            "trg": trg[i * B_SH : (i + 1) * B_SH],
            "wqt": wqt,
            "wkt": wkt,
            "wvt": wvt,
        }
        for i in range(N_CORES)
    ]
    res = run_bass_kernel_spmd(nc, in_maps, list(range(N_CORES)), **kwargs)
    outp = np.concatenate([res.results[i]["out"] for i in range(N_CORES)], axis=0)
    return outp.astype(np.float32), res


def kernel(src, trg, Wq, Wk, Wv):
    outp, _ = _run(src, trg, Wq, Wk, Wv)
    return outp
